# revision 1
# baseline (speedup 1.0000x reference)
"""Multi-head attention Trainium2 kernel (8-core SPMD, no collectives).

Sharding: 8 cores = 4 batches x 2 head-groups (tensor parallel over heads).
Each core receives the full x[b] and the Wq/Wk/Wv/Wo slices for its 8 heads,
computes attention for those heads over ALL 2048 queries, and stores the
PARTIAL output projection (its heads' contribution, fp16, no bias). The host
gather sums the two partials per batch and adds the bias. This removes the
duplicated K/V projection work of a sequence-split sharding: per-core PE work
is 786432 matmul rows (the zero-duplication floor) vs 917504.

Per-core pipeline (matmul inputs bf16, PSUM accumulation fp32):
  A1: V = x @ Wv              -> VP [16 key-blocks, 8 heads, 65] (ones col 64)
  A2: K^T, Q^T per head-pair  -> KT/QT [128 = 2 heads x 64, 4 pb, 2048]
  B:  per (pair, head, qgroup): S^T[k,q] = KT.T QT; P = exp(0.125 S^T);
      acc[65, 512] halves += P^T V' over 16 key blocks (row 64 = denom);
      normalize by reciprocal -> CT [hv, 4 pb, 2048] bf16
  C:  partial_out[q,:] = CT.T @ Wo_slice, per 128-query block, stored fp16.

Schedule: the attention chains (B) are ACT-bound (one exp per kc-step costs
more than the step's two matmuls), so ALL other PE work — A1, A2 for pair
blocks 1-3, and phase C — is split into small "parts" held in a filler queue
and emitted between kc-steps on a hand-tuned schedule (pops[] per chain):
double pops at chain starts cover the acc-ring wait on the previous chain's
normalize; spread pops absorb the per-step ACT slack. The last query half of
the last pair runs as 512-query sub-chains, its out-projection pc0-2
partials are parked in SBUF early, and only a pc-3 matmul + add + store
remain after the final (column-chunked) normalize.
PSUM (8 banks): sc tag 2x2 banks, acc tag 2x1, fill tag (V/KQ/C psums) 2x1.
Softmax skips max-subtraction: scores ~ N(0,1), exp is safe.
"""

import numpy as np
import ml_dtypes

import concourse.bass as bass
import concourse.bacc as bacc
import concourse.mybir as mybir
import concourse.tile as tile

B, S, D = 4, 2048, 1024
H, DQ, DV = 16, 64, 64
P = 128
HPC = H // 2           # heads per core
NPB = HPC // 2         # head-pair blocks per core (2 heads on 128 partitions)
NDC = D // P           # 8 contraction chunks of D
NKB = S // P           # 16 key blocks
NQB = S // P           # 16 query blocks
NQG = 2                # query groups of 1024 (exp chunk)
QG = S // NQG
NCORES = 8
BF16 = mybir.dt.bfloat16
F16 = mybir.dt.float16
F32 = mybir.dt.float32


def build_nc(reps=1):
    nc = bacc.Bacc("TRN2", target_bir_lowering=False, debug=False,
                   num_devices=NCORES)

    # Host supplies partition-major layouts (see make_in_maps below).
    xT = nc.dram_tensor("xT", [P, NKB, NDC, P], BF16, kind="ExternalInput")
    wk = nc.dram_tensor("wk", [NPB, P, NDC, P], BF16, kind="ExternalInput")
    wq = nc.dram_tensor("wq", [NPB, P, NDC, P], BF16, kind="ExternalInput")
    wv = nc.dram_tensor("wv", [P, NDC, HPC * DV], BF16, kind="ExternalInput")
    wo = nc.dram_tensor("wo", [P, NPB, D], BF16, kind="ExternalInput")
    out = nc.dram_tensor("out", [S, D], F16, kind="ExternalOutput")

    Exp = mybir.ActivationFunctionType.Exp

    with tile.TileContext(nc) as tc:
      for _rep in range(reps):
        with (
            tc.tile_pool(name="persist", bufs=1) as persist,
            tc.tile_pool(name="ptp", bufs=3) as ptp,
            tc.tile_pool(name="nrm", bufs=4) as nrmp,
            tc.tile_pool(name="outp", bufs=2) as outp,
            tc.tile_pool(name="ps", bufs=2,
                         space=bass.MemorySpace.PSUM) as ps,
        ):
            KT = persist.tile([P, NPB, S], BF16, tag="KT")
            QT = persist.tile([P, NPB, S], BF16, tag="QT")
            VP = persist.tile([P, NKB, HPC, DV + 1], BF16, tag="VP")
            CT = persist.tile([P, NPB, S], BF16, tag="CT")
            xt = persist.tile([P, NKB, NDC, P], BF16, tag="xt")
            wvt = persist.tile([P, NDC, HPC * DV], BF16, tag="wv")
            wkt = persist.tile([P, NPB, NDC, P], BF16, tag="wk")
            wqt = persist.tile([P, NPB, NDC, P], BF16, tag="wq")
            wot = persist.tile([P, NPB, D], BF16, tag="wo")

            # DMA split across both HWDGE queues: x + K/Q/O weights on the SP
            # queue, wv quarters on the Activation queue (A1 starts earliest;
            # two queues double the serialized issue rate at startup).
            for dc in range(0, NDC, 2):
                nc.scalar.dma_start(wvt[:, dc:dc + 2], wv[:, dc:dc + 2])
            # ones column of V' (softmax denominator accumulator)
            nc.vector.memset(VP[:, :, :, DV:DV + 1], 1.0)
            for kb in range(2):
                nc.sync.dma_start(xt[:, kb], xT[:, kb])
            nc.sync.dma_start(wkt[:, 0], wk[0])
            nc.sync.dma_start(wqt[:, 0], wq[0])
            for kb in range(2, NKB):
                nc.sync.dma_start(xt[:, kb], xT[:, kb])
            for pb in range(1, NPB):
                nc.sync.dma_start(wkt[:, pb], wk[pb])
                nc.sync.dma_start(wqt[:, pb], wq[pb])
            for pb in range(NPB):
                nc.sync.dma_start(wot[:, pb, :], wo[:, pb, :])

            # ---- A1 helper: V projection for one key block (two 4-chunk
            # parts, same shape as the kq parts, usable as PE filler) ----
            def a1_parts(kb):
                st = {}

                def p1():
                    t = ps.tile([P, HPC * DV], F32, tag="fill", name="vps")
                    st["t"] = t
                    for dc in range(4):
                        nc.tensor.matmul(
                            t[:], xt[:, kb, dc, :], wvt[:, dc, :],
                            start=(dc == 0), stop=False)

                def p2():
                    t = st["t"]
                    for dc in range(4, NDC):
                        nc.tensor.matmul(
                            t[:], xt[:, kb, dc, :], wvt[:, dc, :],
                            start=False, stop=(dc == NDC - 1))
                    nc.vector.tensor_copy(
                        VP[:, kb, :, 0:DV],
                        t[:].rearrange("p (h v) -> p h v", h=HPC))

                return [p1, p2]

            # ---- A2 helper: K^T/Q^T projection for one pair block.
            # Emitted as "parts" of 4 contraction chunks each so they can be
            # interleaved between attention kc-steps as PE gap filler. ----
            def kq_parts(pb, nb, wt, Tdst):
                st = {}

                def p1():
                    t = ps.tile([P, 512], F32, tag="fill", name="kq")
                    st["t"] = t
                    for dc in range(4):
                        nc.tensor.matmul(
                            t[:], wt[:, pb, dc, :],
                            xt[:, 4 * nb:4 * nb + 4, dc, :],
                            start=(dc == 0), stop=False)

                def p2():
                    t = st["t"]
                    for dc in range(4, NDC):
                        nc.tensor.matmul(
                            t[:], wt[:, pb, dc, :],
                            xt[:, 4 * nb:4 * nb + 4, dc, :],
                            start=False, stop=(dc == NDC - 1))
                    nc.vector.tensor_copy(
                        Tdst[:, pb, nb * 512:(nb + 1) * 512], t[:])

                return [p1, p2]

            def a2_parts(pb):
                parts = []
                for nb in range(4):
                    parts += kq_parts(pb, nb, wkt, KT)
                for nb in range(4):
                    parts += kq_parts(pb, nb, wqt, QT)
                return parts

            # ---- C helpers: partial out projection. Full parts do all 4
            # pair-block contractions; for the last query group the pc 0-2
            # partial is parked in SBUF (freeing the PSUM fill slot) so only
            # a single pc-3 matmul + add remains after the final normalize ----
            def c_part(qb, half, st):
                def pa():
                    if half == 0:
                        st["o"] = outp.tile([P, D], F16, tag="out",
                                            name="outsb")
                    ops = ps.tile([P, 512], F32, tag="fill", name="ops")
                    st[half] = ops
                    for pc in range(2):
                        nc.tensor.matmul(
                            ops[:],
                            CT[:, pc, qb * P:(qb + 1) * P],
                            wot[:, pc, half * 512:(half + 1) * 512],
                            start=(pc == 0), stop=False)

                def pb_():
                    ops = st[half]
                    for pc in range(2, NPB):
                        nc.tensor.matmul(
                            ops[:],
                            CT[:, pc, qb * P:(qb + 1) * P],
                            wot[:, pc, half * 512:(half + 1) * 512],
                            start=False, stop=(pc == NPB - 1))
                    nc.vector.tensor_copy(
                        st["o"][:, half * 512:(half + 1) * 512], ops[:])
                    if half == 1:
                        nc.sync.dma_start(
                            out[qb * P:(qb + 1) * P, :], st["o"][:])
                return [pa, pb_]

            def c_parts(qg):
                parts = []
                for qb in range(qg * (NQB // NQG), (qg + 1) * (NQB // NQG)):
                    st = {}
                    parts += c_part(qb, 0, st)
                    parts += c_part(qb, 1, st)
                return parts

            ctmps = {}

            def c_park_part(qb, half):
                tmp = outp.tile([P, 512], F32, tag="ctmp", bufs=16,
                                name="ctmp")
                ctmps[(qb, half)] = tmp

                def p():
                    ops = ps.tile([P, 512], F32, tag="fill", name="ops")
                    for pc in range(NPB - 1):
                        nc.tensor.matmul(
                            ops[:],
                            CT[:, pc, qb * P:(qb + 1) * P],
                            wot[:, pc, half * 512:(half + 1) * 512],
                            start=(pc == 0), stop=(pc == NPB - 2))
                    nc.vector.tensor_copy(tmp[:], ops[:])
                return p

            def c_finish_part(qb, half):
                def p():
                    ops = ps.tile([P, 512], F32, tag="fill", name="ops")
                    nc.tensor.matmul(
                        ops[:],
                        CT[:, NPB - 1, qb * P:(qb + 1) * P],
                        wot[:, NPB - 1, half * 512:(half + 1) * 512],
                        start=True, stop=True)
                    o = outp.tile([P, 512], F16, tag="outf", bufs=6,
                                  name="outsf")
                    nc.vector.tensor_add(o[:], ops[:],
                                         ctmps[(qb, half)][:])
                    nc.sync.dma_start(
                        out[qb * P:(qb + 1) * P,
                            half * 512:(half + 1) * 512], o[:])
                return p

            # ---- B: attention chains; everything else (V projection, K/Q
            # projections for pb1-3, phase C) is a filler queue feeding the
            # PE between kc-steps — covering chain-boundary acc stalls and
            # the ACT-paced slack — in dependency-safe order ----
            from collections import deque
            fillq = deque()
            a1p = {kb: a1_parts(kb) for kb in range(NKB)}
            fillq.extend(kq_parts(0, 1, wkt, KT))
            fillq.extend(kq_parts(0, 2, wkt, KT))
            fillq.extend(a1p[1])
            fillq.extend(kq_parts(0, 3, wkt, KT))
            for kb in range(2, NKB):
                fillq.extend(a1p[kb])
            fillq.extend(kq_parts(0, 2, wqt, QT))
            fillq.extend(kq_parts(0, 3, wqt, QT))
            for pb in range(1, NPB):
                fillq.extend(a2_parts(pb))

            def chain(pb, h, q0, qw, pops, last=False):
                """One attention chain: queries [q0, q0+qw) for head h of
                pair block pb. Filler pops sit between exp and the PV
                matmuls so they cover the acc-ring wait at chain start.
                pops[kc] = number of filler parts to emit at that step."""
                hh = (h % 2) * 64
                gw = 512
                ng = qw // gw
                accs = [ps.tile([DV + 1, gw], F32, tag="acc",
                                name=f"acc{g}") for g in range(ng)]

                def emit_sc(kc):
                    sc = ps.tile([P, qw], F32, tag="sc", name="sc")
                    for half in range(ng):
                        nc.tensor.matmul(
                            sc[:, half * gw:(half + 1) * gw],
                            KT[hh:hh + 64, pb, kc * P:(kc + 1) * P],
                            QT[hh:hh + 64, pb,
                               q0 + half * gw:q0 + (half + 1) * gw],
                            start=True, stop=True)
                    return sc

                # kc loop software-pipelined by one step: scores(kc+1) are
                # emitted before PV(kc) so the exp stream never waits on a
                # just-written sc tile (kills a ~144ns/kc ACT bubble)
                sc_prev = emit_sc(0)
                for kc1 in range(1, NKB + 1):
                    kc = kc1 - 1
                    sc_cur = emit_sc(kc1) if kc1 < NKB else None
                    pt = ptp.tile([P, qw], BF16, tag="pt", name="pt")
                    nc.scalar.activation(pt[:], sc_prev[:], Exp, scale=0.125)
                    for _ in range(pops[kc]):
                        if fillq:
                            fillq.popleft()()
                    for g in range(ng):
                        nc.tensor.matmul(
                            accs[g][:],
                            VP[:, kc, h, :],
                            pt[:, g * gw:(g + 1) * gw],
                            start=(kc == 0), stop=(kc == NKB - 1))
                    sc_prev = sc_cur
                for g in range(ng):
                    if last:
                        # chunked normalize: each 128-col piece unblocks one
                        # query block's out-projection finish in the tail
                        for ch in range(qw // P):
                            cs = slice(ch * P, (ch + 1) * P)
                            rec = nrmp.tile([1, P], F32, tag="recs",
                                            name="recs")
                            nc.vector.reciprocal(
                                rec[:], accs[g][DV:DV + 1, cs])
                            bc = nrmp.tile([DV, P], F32, tag="bcs",
                                           name="bcs")
                            nc.gpsimd.partition_broadcast(bc[:], rec[:])
                            nc.vector.tensor_mul(
                                CT[hh:hh + 64, pb, q0 + ch * P:
                                   q0 + (ch + 1) * P],
                                accs[g][0:DV, cs], bc[:])
                        continue
                    if False:
                        pass
                    else:
                        # one copy is the only acc reader: frees the PSUM
                        # ring slot ~1.7us earlier than recip+bcast+mul
                        # would, killing the next chain's first-PV stall
                        src = nrmp.tile([DV + 1, gw], F32, tag="cacc",
                                        name="cacc")
                        nc.vector.tensor_copy(src[:], accs[g][:])
                    rec = nrmp.tile([1, gw], F32, tag="rec", name="rec")
                    nc.vector.reciprocal(rec[:], src[DV:DV + 1, :])
                    bc = nrmp.tile([DV, gw], F32, tag="bc", name="bc")
                    nc.gpsimd.partition_broadcast(bc[:], rec[:])
                    nc.vector.tensor_mul(
                        CT[hh:hh + 64, pb,
                           q0 + g * gw:q0 + (g + 1) * gw],
                        src[0:DV, :], bc[:])

            # chain sequence: pb0-2 and pb3-qg0 full-width; pb3-qg1 as
            # 512-query sub-chains so the out-projection finish for its
            # first half overlaps the second half's chains
            def kcs(*idxs):
                return [1 if kc in idxs else 0 for kc in range(NKB)]

            EVERY = [1] * NKB
            seq = []
            for pb in range(NPB - 1):
                for qg in range(NQG):
                    for h in (2 * pb, 2 * pb + 1):
                        seq.append((pb, h, qg * QG, QG, None))
            for h in (2 * NPB - 2, 2 * NPB - 1):
                seq.append((NPB - 1, h, 0, QG,
                            [1, 1, 0, 1, 0, 1, 0, 1, 0, 1, 0, 1, 0, 1, 0,
                             0]))
            subs = [(QG, 512, [[0, 0] + [1] * (NKB - 2),
                                [1, 0, 1, 1, 0, 1, 1, 0, 1, 1, 0, 1, 1,
                                 0, 0, 0]]),
                    (QG + 512, 512, [kcs(*range(2, NKB)), EVERY])]
            for q0s, qws, popsl in subs:
                for h, pops in zip((2 * NPB - 2, 2 * NPB - 1), popsl):
                    seq.append((NPB - 1, h, q0s, qws, pops))
            seq[0] = seq[0][:4] + ([5, 5, 3, 3] + [2] * 12,)
            # double pop at kc0 covers the acc-ring wait on the previous
            # chain's normalize; kc8 pop covers mid-chain ACT-pace slack
            # (a pop's benefit drains at ~185ns/kc, reaching ~4-5 steps)
            mid4 = [2, 1, 0, 0, 0, 0, 0, 1, 0, 0, 0, 0, 0, 0, 0, 0]
            mid5 = [1, 1, 0, 0, 1, 0, 0, 1, 0, 0, 0, 1, 0, 0, 0, 0]
            for i in range(1, 3):
                seq[i] = seq[i][:4] + (mid4,)
            for i in range(3, 12):
                seq[i] = seq[i][:4] + (mid5,)

            # pre-B direct work: V proj of key block 0 and the pb0 K/Q
            # projections needed by the first chain's early kc steps
            for part in a1p[0]:
                part()
            for part in (kq_parts(0, 0, wkt, KT) + kq_parts(0, 0, wqt, QT)
                         + kq_parts(0, 1, wqt, QT)):
                part()

            for idx, (pb, h, q0, qw, pops) in enumerate(seq):
                chain(pb, h, q0, qw, pops, last=(idx == len(seq) - 1))
                if idx == 11:
                    # pb0-2 CTs complete: queue the pc 0-2 partials of the
                    # last query group's out projection (parked in SBUF).
                    fillq.extend(c_park_part(qb, half)
                                 for qb in range(NQB // NQG, NQB)
                                 for half in range(2))
                if idx == 13:
                    # pb3 qg0 chains + normalize done: queue phase C for
                    # qg0 to interleave with the qg1 sub-chains.
                    fillq.extend(c_parts(0))
                if idx == 15:
                    # first-half qg1 CTs complete: queue its finish parts.
                    fillq.extend(c_finish_part(qb, half)
                                 for qb in range(8, 12)
                                 for half in range(2))
            # flush leftover filler, then finish the last queries (tail:
            # one pc-3 matmul + add per (qb, half) on the parked partials)
            while fillq:
                fillq.popleft()()
            for qb in range(12, NQB):
                for half in range(2):
                    c_finish_part(qb, half)()

    nc.compile()
    return nc


def make_in_maps(x, Wq, Wk, Wv, Wo, bo):
    bf = ml_dtypes.bfloat16
    x = np.asarray(x, np.float32)

    def xmajor(xb):  # [S, D] -> [P, NKB, NDC, P]
        return np.ascontiguousarray(
            xb.T.reshape(NDC, P, NKB, P).transpose(1, 2, 0, 3)).astype(bf)

    def wpairs(W, hg):  # [H, D, 64] -> core slice [NPB, P, NDC, P]
        a = (np.asarray(W, np.float32)[hg * HPC:(hg + 1) * HPC]
             .transpose(1, 0, 2).reshape(D, HPC * 64))
        return np.ascontiguousarray(
            a.reshape(NDC, P, NPB, P).transpose(2, 1, 0, 3)).astype(bf)

    def pm(a):  # [D, N] -> partition-major [P, NDC, N]
        return np.ascontiguousarray(
            a.reshape(NDC, P, a.shape[1]).transpose(1, 0, 2)).astype(bf)

    xT_b = [xmajor(x[b]) for b in range(B)]
    Wv_f = np.asarray(Wv, np.float32)
    Wo_f = np.asarray(Wo, np.float32)

    in_maps = []
    for c in range(NCORES):
        b, hg = c // 2, c % 2
        wv_h = pm(Wv_f[hg * HPC:(hg + 1) * HPC]
                  .transpose(1, 0, 2).reshape(D, HPC * DV))
        wo_h = np.ascontiguousarray(
            Wo_f[hg * HPC * DV:(hg + 1) * HPC * DV]
            .reshape(NPB, P, D).transpose(1, 0, 2)).astype(bf)
        in_maps.append({
            "xT": xT_b[b],
            "wk": wpairs(Wk, hg),
            "wq": wpairs(Wq, hg),
            "wv": wv_h,
            "wo": wo_h,
        })
    return in_maps


def kernel(x, Wq, Wk, Wv, Wo, bo):
    from concourse.bass_utils import run_bass_kernel_spmd
    in_maps = make_in_maps(x, Wq, Wk, Wv, Wo, bo)
    nc = build_nc()
    res = run_bass_kernel_spmd(nc, in_maps, list(range(NCORES))).results
    bo_f = np.asarray(bo, np.float32)
    full = np.empty((B, S, D), np.float32)
    for b in range(B):
        full[b] = (np.asarray(res[2 * b]["out"], np.float32)
                   + np.asarray(res[2 * b + 1]["out"], np.float32)
                   + bo_f)
    return full



# revision 13
# speedup vs baseline: 1.0114x; 1.0114x over previous
"""Multi-head attention Trainium2 kernel (8-core SPMD, no collectives).

Sharding: 8 cores = 4 batches x 2 head-groups (tensor parallel over heads).
Each core receives the full x[b] and the Wq/Wk/Wv/Wo slices for its 8 heads,
computes attention for those heads over ALL 2048 queries, and stores the
PARTIAL output projection (its heads' contribution, fp16, no bias). The host
gather sums the two partials per batch and adds the bias.

Numerics (matmul inputs fp16, PSUM accumulation fp32, PV in fp8 DoubleRow):
  A1: V = x @ Wv; V quantized to fp8e4m3 as V1 + residual V2 (split
      compensation), stored [16 kb, 8 heads, 66] (ones col 64 in V1 only,
      col 65 = pad for the 16B DoubleRow stride alignment)
  A2: K^T, Q^T per head-pair -> KT/QT fp16 [128 = 2 heads x 64, 4 pb, 2048]
  B:  per (pair, head, qgroup): S^T[k,q] = KT.T QT (fp16, psum fp32);
      P~ = exp(0.125 S^T - 6 ln2) -> fp8e4m3 pair tiles [128, 2, 1024]
      (2^-6 scale keeps max exp ~63 << 240 = e4m3 max; the softmax ratio
      cancels the scale); acc[65, 512] halves += V'.T P~ as DoubleRow
      matmuls over kc PAIRS (contraction 256, 0.5 cyc/row) for V1 and V2;
      row 64 = denom; normalize by reciprocal -> CT fp16
  C:  partial_out[q,:] = CT.T @ Wo_slice per 128-query block, fp16.

Schedule: chains are ACT-paced (~1038ns/kc exp vs ~640ns/kc chain PE work);
all other PE work (A1, A2 pb1-3, C) is a filler queue popped between kc
steps. PE total 655360 cyc (273us) vs ACT 265.7us. PSUM: sc 2x2 banks,
acc 2x1, fill 2x1. Softmax skips max-subtraction: scores ~ N(0,1) with
|s|max ~ 8.3, exp(s)*2^-6 fits e4m3.
"""

import numpy as np
import ml_dtypes

import concourse.bass as bass
import concourse.bacc as bacc
import concourse.mybir as mybir
import concourse.tile as tile

B, S, D = 4, 2048, 1024
H, DQ, DV = 16, 64, 64
P = 128
HPC = H // 2           # heads per core
NPB = HPC // 2         # head-pair blocks per core (2 heads on 128 partitions)
NDC = D // P           # 8 contraction chunks of D
NKB = S // P           # 16 key blocks
NQB = S // P           # 16 query blocks
NQG = 2                # query groups of 1024 (exp chunk)
QG = S // NQG
NCORES = 8
F16 = mybir.dt.float16
F8 = mybir.dt.float8e4
F32 = mybir.dt.float32
DVP = DV + 2           # V' row: 64 v cols + ones col + pad (16B stride align)
PBIAS = -6 * float(np.log(2.0))   # exp scale 2^-6
# kc pairs computed in fp8 DoubleRow; the rest run plain fp16 PV. Half/half
# keeps the end-to-end rel err ~1.4e-2 on HW (fp8 P-noise scales as sqrt(f)).
FP8_PAIRS = frozenset((0, 2, 4, 6))


def build_nc(reps=1):
    nc = bacc.Bacc("TRN2", target_bir_lowering=False, debug=False,
                   num_devices=NCORES)

    # Host supplies partition-major layouts (see make_in_maps below).
    xT = nc.dram_tensor("xT", [P, NKB, NDC, P], F16, kind="ExternalInput")
    wk = nc.dram_tensor("wk", [NPB, P, NDC, P], F16, kind="ExternalInput")
    wq = nc.dram_tensor("wq", [NPB, P, NDC, P], F16, kind="ExternalInput")
    wv = nc.dram_tensor("wv", [P, NDC, HPC * DV], F16, kind="ExternalInput")
    wo = nc.dram_tensor("wo", [P, NPB, D], F16, kind="ExternalInput")
    out = nc.dram_tensor("out", [S, D], F16, kind="ExternalOutput")

    Exp = mybir.ActivationFunctionType.Exp
    DR = mybir.MatmulPerfMode.DoubleRow

    with tile.TileContext(nc) as tc:
      for _rep in range(reps):
        with (
            tc.tile_pool(name="persist", bufs=1) as persist,
            tc.tile_pool(name="ptp", bufs=3) as ptp,
            tc.tile_pool(name="nrm", bufs=4) as nrmp,
            tc.tile_pool(name="outp", bufs=2) as outp,
            tc.tile_pool(name="ps", bufs=2,
                         space=bass.MemorySpace.PSUM) as ps,
        ):
            KT = persist.tile([P, NPB, S], F16, tag="KT")
            QT = persist.tile([P, NPB, S], F16, tag="QT")
            # compact V' storage: fp8 split pair-blocks and fp16 blocks
            # each hold only their half of the key blocks.
            # kb -> compact index: (kb // 4) * 2 + kb % 2 (pairs adjacent)
            VP1 = persist.tile([P, NKB // 2, HPC, DVP], F8, tag="VP1")
            VP2 = persist.tile([P, NKB // 2, HPC, DVP], F8, tag="VP2")
            VPf = persist.tile([P, NKB // 2, HPC, DV + 1], F16, tag="VPf")
            CT = persist.tile([P, NPB, S], F16, tag="CT")
            xt = persist.tile([P, NKB, NDC, P], F16, tag="xt")
            wvt = persist.tile([P, NDC, HPC * DV], F16, tag="wv")
            wkt = persist.tile([P, NPB, NDC, P], F16, tag="wk")
            wqt = persist.tile([P, NPB, NDC, P], F16, tag="wq")
            wot = persist.tile([P, NPB, D], F16, tag="wo")
            pbias = persist.tile([P, 1], F32, tag="pbias")
            nc.vector.memset(pbias[:], PBIAS)

            # DMA split across both HWDGE queues: x + K/Q/O weights on the SP
            # queue, wv quarters on the Activation queue (A1 starts earliest;
            # two queues double the serialized issue rate at startup).
            for dc in range(0, NDC, 2):
                nc.scalar.dma_start(wvt[:, dc:dc + 2], wv[:, dc:dc + 2])
            # ones column of V' (softmax denominator accumulator); V2 zeros
            nc.vector.memset(VP1[:, :, :, DV:DV + 1], 1.0)
            nc.vector.memset(VP2[:, :, :, DV:DV + 1], 0.0)
            nc.vector.memset(VPf[:, :, :, DV:DV + 1], 1.0)
            for kb in range(2):
                nc.sync.dma_start(xt[:, kb], xT[:, kb])
            nc.sync.dma_start(wkt[:, 0], wk[0])
            nc.sync.dma_start(wqt[:, 0], wq[0])
            for kb in range(2, NKB):
                nc.sync.dma_start(xt[:, kb], xT[:, kb])
            for pb in range(1, NPB):
                nc.sync.dma_start(wkt[:, pb], wk[pb])
                nc.sync.dma_start(wqt[:, pb], wq[pb])
            for pb in range(NPB):
                nc.sync.dma_start(wot[:, pb, :], wo[:, pb, :])

            # ---- A1 helper: V projection for one key block; the psum V is
            # split-quantized into fp8 V1 + residual V2 ----
            def a1_parts(kb):
                st = {}

                def p1():
                    t = ps.tile([P, HPC * DV], F32, tag="fill", name="vps")
                    st["t"] = t
                    for dc in range(4):
                        nc.tensor.matmul(
                            t[:], xt[:, kb, dc, :], wvt[:, dc, :],
                            start=(dc == 0), stop=False)

                def p2():
                    t = st["t"]
                    for dc in range(4, NDC):
                        nc.tensor.matmul(
                            t[:], xt[:, kb, dc, :], wvt[:, dc, :],
                            start=False, stop=(dc == NDC - 1))
                    tv = t[:].rearrange("p (h v) -> p h v", h=HPC)
                    ci = (kb // 4) * 2 + kb % 2
                    if kb // 2 in FP8_PAIRS:
                        nc.vector.tensor_copy(VP1[:, ci, :, 0:DV], tv)
                        # residual: V2 = V - fp8(V) (fp8 output quantizes)
                        nc.vector.scalar_tensor_tensor(
                            VP2[:, ci, :, 0:DV], VP1[:, ci, :, 0:DV], -1.0,
                            tv, mybir.AluOpType.mult, mybir.AluOpType.add)
                    else:
                        nc.vector.tensor_copy(VPf[:, ci, :, 0:DV], tv)

                return [p1, p2]

            # ---- A2 helper: K^T/Q^T projection for one pair block ----
            def kq_parts(pb, nb, wt, Tdst):
                st = {}

                def p1():
                    t = ps.tile([P, 512], F32, tag="fill", name="kq")
                    st["t"] = t
                    for dc in range(4):
                        nc.tensor.matmul(
                            t[:], wt[:, pb, dc, :],
                            xt[:, 4 * nb:4 * nb + 4, dc, :],
                            start=(dc == 0), stop=False)

                def p2():
                    t = st["t"]
                    for dc in range(4, NDC):
                        nc.tensor.matmul(
                            t[:], wt[:, pb, dc, :],
                            xt[:, 4 * nb:4 * nb + 4, dc, :],
                            start=False, stop=(dc == NDC - 1))
                    nc.vector.tensor_copy(
                        Tdst[:, pb, nb * 512:(nb + 1) * 512], t[:])

                return [p1, p2]

            def a2_parts(pb):
                parts = []
                for nb in range(4):
                    parts += kq_parts(pb, nb, wkt, KT)
                for nb in range(4):
                    parts += kq_parts(pb, nb, wqt, QT)
                return parts

            # ---- C helpers: partial out projection ----
            def c_part(qb, half, st):
                def pa():
                    if half == 0:
                        st["o"] = outp.tile([P, D], F16, tag="out",
                                            name="outsb")
                    ops = ps.tile([P, 512], F32, tag="fill", name="ops")
                    st[half] = ops
                    for pc in range(2):
                        nc.tensor.matmul(
                            ops[:],
                            CT[:, pc, qb * P:(qb + 1) * P],
                            wot[:, pc, half * 512:(half + 1) * 512],
                            start=(pc == 0), stop=False)

                def pb_():
                    ops = st[half]
                    for pc in range(2, NPB):
                        nc.tensor.matmul(
                            ops[:],
                            CT[:, pc, qb * P:(qb + 1) * P],
                            wot[:, pc, half * 512:(half + 1) * 512],
                            start=False, stop=(pc == NPB - 1))
                    nc.vector.tensor_copy(
                        st["o"][:, half * 512:(half + 1) * 512], ops[:])
                    if half == 1:
                        nc.sync.dma_start(
                            out[qb * P:(qb + 1) * P, :], st["o"][:])
                return [pa, pb_]

            def c_parts(qg):
                parts = []
                for qb in range(qg * (NQB // NQG), (qg + 1) * (NQB // NQG)):
                    st = {}
                    parts += c_part(qb, 0, st)
                    parts += c_part(qb, 1, st)
                return parts

            ctmps = {}

            def c_park_part(qb, half):
                tmp = outp.tile([P, 512], F16, tag="ctmp", bufs=16,
                                name="ctmp")
                ctmps[(qb, half)] = tmp

                def p():
                    ops = ps.tile([P, 512], F32, tag="fill", name="ops")
                    for pc in range(NPB - 1):
                        nc.tensor.matmul(
                            ops[:],
                            CT[:, pc, qb * P:(qb + 1) * P],
                            wot[:, pc, half * 512:(half + 1) * 512],
                            start=(pc == 0), stop=(pc == NPB - 2))
                    nc.vector.tensor_copy(tmp[:], ops[:])
                return p

            def c_finish_part(qb, half):
                def p():
                    ops = ps.tile([P, 512], F32, tag="fill", name="ops")
                    nc.tensor.matmul(
                        ops[:],
                        CT[:, NPB - 1, qb * P:(qb + 1) * P],
                        wot[:, NPB - 1, half * 512:(half + 1) * 512],
                        start=True, stop=True)
                    o = outp.tile([P, 512], F16, tag="outf", bufs=6,
                                  name="outsf")
                    nc.vector.tensor_add(o[:], ops[:],
                                         ctmps[(qb, half)][:])
                    nc.sync.dma_start(
                        out[qb * P:(qb + 1) * P,
                            half * 512:(half + 1) * 512], o[:])
                return p

            # ---- B: attention chains; filler queue feeds the PE between
            # kc-steps in dependency-safe order ----
            from collections import deque
            fillq = deque()
            a1p = {kb: a1_parts(kb) for kb in range(NKB)}
            fillq.extend(kq_parts(0, 1, wkt, KT))
            fillq.extend(kq_parts(0, 2, wkt, KT))
            fillq.extend(a1p[1])
            fillq.extend(kq_parts(0, 3, wkt, KT))
            for kb in range(2, NKB):
                fillq.extend(a1p[kb])
            fillq.extend(kq_parts(0, 2, wqt, QT))
            fillq.extend(kq_parts(0, 3, wqt, QT))
            for pb in range(1, NPB):
                fillq.extend(a2_parts(pb))

            def chain(pb, h, q0, qw, pops, last=False):
                """One attention chain: queries [q0, q0+qw) for head h of
                pair block pb. pops[kc] = filler parts popped at step kc."""
                hh = (h % 2) * 64
                gw = 512
                ng = qw // gw
                accs = [ps.tile([DV + 1, gw], F32, tag="acc",
                                name=f"acc{g}") for g in range(ng)]

                def emit_sc(kc):
                    sc = ps.tile([P, qw], F32, tag="sc", name="sc")
                    for half in range(ng):
                        nc.tensor.matmul(
                            sc[:, half * gw:(half + 1) * gw],
                            KT[hh:hh + 64, pb, kc * P:(kc + 1) * P],
                            QT[hh:hh + 64, pb,
                               q0 + half * gw:q0 + (half + 1) * gw],
                            start=True, stop=True)
                    return sc

                def emit_pv(kp, pt):
                    # DoubleRow PV over the kc pair (2*kp, 2*kp+1):
                    # acc[g] 256-col halves += V'.T @ P~ for V1 then V2
                    for g in range(ng):
                        for hf in range(0, gw, 256):
                            col = g * gw + hf
                            for t, VPt in ((0, VP1), (1, VP2)):
                                # start/stop once per acc tile: the PSUM
                                # zero region spans the whole bank
                                kpi = kp // 2
                                nc.tensor.matmul(
                                    accs[g][:, hf:hf + 256],
                                    VPt[:, 2 * kpi:2 * kpi + 2,
                                        h % HPC, 0:DV + 1],
                                    pt[:, :, col:col + 256],
                                    start=(kp == 0 and t == 0 and hf == 0),
                                    stop=(kp == NKB // 2 - 1 and t == 1
                                          and hf == gw - 256),
                                    perf_mode=DR)

                def emit_pv16(kc, pt):
                    # plain fp16 PV for one kc block (pair 0 is fp8, so the
                    # group start always comes from the first DR matmul)
                    for g in range(ng):
                        nc.tensor.matmul(
                            accs[g][:],
                            VPf[:, (kc // 4) * 2 + kc % 2, h % HPC, :],
                            pt[:, g * gw:(g + 1) * gw],
                            start=False, stop=(kc == NKB - 1))

                # kc loop software-pipelined by one step: scores(kc+1)
                # emitted before exp(kc)'s consumers so the exp stream never
                # waits on a just-written sc tile
                sc_prev = emit_sc(0)
                pt_pair = None
                for kc1 in range(1, NKB + 1):
                    kc = kc1 - 1
                    sc_cur = emit_sc(kc1) if kc1 < NKB else None
                    fp8_pair = (kc // 2) in FP8_PAIRS
                    if fp8_pair:
                        if kc % 2 == 0:
                            pt_pair = ptp.tile([P, 2, qw], F8, tag="pt",
                                               bufs=2, name="pt")
                        nc.scalar.activation(pt_pair[:, kc % 2, :],
                                             sc_prev[:], Exp,
                                             scale=0.125, bias=pbias[:])
                        ptf = None
                    else:
                        ptf = ptp.tile([P, qw], F16, tag="ptf", name="ptf")
                        nc.scalar.activation(ptf[:], sc_prev[:], Exp,
                                             scale=0.125, bias=pbias[:])
                    for _ in range(pops[kc]):
                        if fillq:
                            fillq.popleft()()
                    if fp8_pair:
                        if kc % 2 == 1:
                            emit_pv(kc // 2, pt_pair)
                    else:
                        emit_pv16(kc, ptf)
                    sc_prev = sc_cur
                for g in range(ng):
                    if last:
                        # chunked normalize: each 128-col piece unblocks one
                        # query block's out-projection finish in the tail
                        for ch in range(qw // P):
                            cs = slice(ch * P, (ch + 1) * P)
                            rec = nrmp.tile([1, P], F32, tag="recs",
                                            name="recs")
                            nc.vector.reciprocal(
                                rec[:], accs[g][DV:DV + 1, cs])
                            bc = nrmp.tile([DV, P], F32, tag="bcs",
                                           name="bcs")
                            nc.gpsimd.partition_broadcast(bc[:], rec[:])
                            nc.vector.tensor_mul(
                                CT[hh:hh + 64, pb, q0 + ch * P:
                                   q0 + (ch + 1) * P],
                                accs[g][0:DV, cs], bc[:])
                        continue
                    # one copy is the only acc reader: frees the PSUM ring
                    # slot early, killing the next chain's first-PV stall
                    src = nrmp.tile([DV + 1, gw], F32, tag="cacc",
                                    name="cacc")
                    nc.vector.tensor_copy(src[:], accs[g][:])
                    rec = nrmp.tile([1, gw], F32, tag="rec", name="rec")
                    nc.vector.reciprocal(rec[:], src[DV:DV + 1, :])
                    bc = nrmp.tile([DV, gw], F32, tag="bc", name="bc")
                    nc.gpsimd.partition_broadcast(bc[:], rec[:])
                    nc.vector.tensor_mul(
                        CT[hh:hh + 64, pb,
                           q0 + g * gw:q0 + (g + 1) * gw],
                        src[0:DV, :], bc[:])

            # chain sequence: pb0-2 and pb3-qg0 full-width; pb3-qg1 as
            # 512-query sub-chains so the out-projection finish for its
            # first half overlaps the second half's chains
            def kcs(*idxs):
                return [1 if kc in idxs else 0 for kc in range(NKB)]

            EVERY = [1] * NKB
            seq = []
            for pb in range(NPB - 1):
                for qg in range(NQG):
                    for h in (2 * pb, 2 * pb + 1):
                        seq.append((pb, h, qg * QG, QG, None))
            for h in (2 * NPB - 2, 2 * NPB - 1):
                seq.append((NPB - 1, h, 0, QG,
                            [1, 1, 0, 1, 0, 1, 0, 1, 0, 1, 0, 1, 0, 1, 0,
                             0]))
            subs = [(QG, 512, [[0, 0] + [1] * (NKB - 2),
                                [1, 0, 1, 1, 0, 1, 1, 0, 1, 1, 0, 1, 1,
                                 0, 0, 0]]),
                    (QG + 512, 512, [kcs(*range(2, NKB)), EVERY])]
            for q0s, qws, popsl in subs:
                for h, pops in zip((2 * NPB - 2, 2 * NPB - 1), popsl):
                    seq.append((NPB - 1, h, q0s, qws, pops))
            seq[0] = seq[0][:4] + ([5, 5, 3, 3] + [2] * 12,)
            # double pop at kc0 covers the acc-ring wait on the previous
            # chain's normalize; spread pops absorb per-step ACT slack
            mid4 = [2, 1, 1, 0, 1, 0, 1, 0, 1, 0, 1, 0, 1, 0, 0, 0]
            mid5 = [2, 1, 0, 1, 0, 1, 0, 1, 0, 1, 0, 1, 0, 1, 0, 0]
            for i in range(1, 3):
                seq[i] = seq[i][:4] + (mid4,)
            for i in range(3, 12):
                seq[i] = seq[i][:4] + (mid5,)

            # pre-B direct work: V proj of key block 0 and the pb0 K/Q
            # projections needed by the first chain's early kc steps
            for part in a1p[0]:
                part()
            for part in (kq_parts(0, 0, wkt, KT) + kq_parts(0, 0, wqt, QT)
                         + kq_parts(0, 1, wqt, QT)):
                part()

            for idx, (pb, h, q0, qw, pops) in enumerate(seq):
                chain(pb, h, q0, qw, pops, last=(idx == len(seq) - 1))
                if idx == 11:
                    # pb0-2 CTs complete: queue the pc 0-2 partials of the
                    # last query group's out projection (parked in SBUF).
                    fillq.extend(c_park_part(qb, half)
                                 for qb in range(NQB // NQG, NQB)
                                 for half in range(2))
                if idx == 13:
                    # pb3 qg0 chains + normalize done: queue phase C for
                    # qg0 to interleave with the qg1 sub-chains.
                    fillq.extend(c_parts(0))
                if idx == 15:
                    # first-half qg1 CTs complete: queue its finish parts.
                    fillq.extend(c_finish_part(qb, half)
                                 for qb in range(8, 12)
                                 for half in range(2))
            # flush leftover filler, then finish the last queries (tail:
            # one pc-3 matmul + add per (qb, half) on the parked partials)
            while fillq:
                fillq.popleft()()
            for qb in range(12, NQB):
                for half in range(2):
                    c_finish_part(qb, half)()

    nc.compile()
    return nc


def make_in_maps(x, Wq, Wk, Wv, Wo, bo):
    f16 = np.float16
    x = np.asarray(x, np.float32)

    def xmajor(xb):  # [S, D] -> [P, NKB, NDC, P]
        return np.ascontiguousarray(
            xb.T.reshape(NDC, P, NKB, P).transpose(1, 2, 0, 3)).astype(f16)

    def wpairs(W, hg):  # [H, D, 64] -> core slice [NPB, P, NDC, P]
        a = (np.asarray(W, np.float32)[hg * HPC:(hg + 1) * HPC]
             .transpose(1, 0, 2).reshape(D, HPC * 64))
        return np.ascontiguousarray(
            a.reshape(NDC, P, NPB, P).transpose(2, 1, 0, 3)).astype(f16)

    def pm(a):  # [D, N] -> partition-major [P, NDC, N]
        return np.ascontiguousarray(
            a.reshape(NDC, P, a.shape[1]).transpose(1, 0, 2)).astype(f16)

    xT_b = [xmajor(x[b]) for b in range(B)]
    Wv_f = np.asarray(Wv, np.float32)
    Wo_f = np.asarray(Wo, np.float32)

    in_maps = []
    for c in range(NCORES):
        b, hg = c // 2, c % 2
        wv_h = pm(Wv_f[hg * HPC:(hg + 1) * HPC]
                  .transpose(1, 0, 2).reshape(D, HPC * DV))
        wo_h = np.ascontiguousarray(
            Wo_f[hg * HPC * DV:(hg + 1) * HPC * DV]
            .reshape(NPB, P, D).transpose(1, 0, 2)).astype(f16)
        in_maps.append({
            "xT": xT_b[b],
            "wk": wpairs(Wk, hg),
            "wq": wpairs(Wq, hg),
            "wv": wv_h,
            "wo": wo_h,
        })
    return in_maps


def kernel(x, Wq, Wk, Wv, Wo, bo):
    from concourse.bass_utils import run_bass_kernel_spmd
    in_maps = make_in_maps(x, Wq, Wk, Wv, Wo, bo)
    nc = build_nc()
    res = run_bass_kernel_spmd(nc, in_maps, list(range(NCORES))).results
    bo_f = np.asarray(bo, np.float32)
    full = np.empty((B, S, D), np.float32)
    for b in range(B):
        full[b] = (np.asarray(res[2 * b]["out"], np.float32)
                   + np.asarray(res[2 * b + 1]["out"], np.float32)
                   + bo_f)
    return full
